# revision 1
# baseline (speedup 1.0000x reference)
"""GCN message-passing kernel for 8 Trainium2 NeuronCores.

out = log_softmax(mean_agg(norm * (x@W)[src] -> dst) + b)

Strategy (graph/data parallel per the sharding hint):
  - Shard dst nodes (and their incoming edges) across 8 cores; within a
    core, assign dst nodes to 128-lane blocks with LPT balancing so each
    block has ~equal edge count (host permutes rows, un-permutes output).
  - Phase A: each core computes xw = x_shard @ W (PE, bf16), scales by
    deg^-1/2 into y (bf16), stores its y shard to DRAM.
  - Phase B: AllGather the y shards so every core has the full y table.
  - Phase C: per dst block, gather y[src] rows via indirect DMA, build
    one-hot(dst_lane) with DVE is_equal against a replicated iota, and
    aggregate with PE matmuls accumulating in PSUM.  Epilogue applies
    deg^-3/2 scaling, the self-loop term, bias, and log_softmax.

Math identity used (self-loops make deg >= 1 and cnt == deg):
  out[d] = deg[d]^-3/2 * (sum_{e: dst=d} y[src_e] + y[d]) + b
  with y[n] = xw[n] * deg[n]^-1/2, followed by row log_softmax.
"""

import heapq
from contextlib import ExitStack

import numpy as np
import ml_dtypes

import concourse.bacc as bacc
import concourse.bass as bass
import concourse.mybir as mybir
import concourse.tile as tile
from concourse import bass_utils
from concourse.bass import IndirectOffsetOnAxis

# Problem sizes (hardcoded per the harness contract).
N = 100000
F = 256
C = 64
E = 3200000
N_CORES = 8
NSH = N // N_CORES          # 12500 dst nodes per core
PB = (NSH + 127) // 128     # 98 blocks of 128 dst nodes
NP = PB * 128               # padded shard rows (12544)

f32 = mybir.dt.float32
bf16 = mybir.dt.bfloat16
i32 = mybir.dt.int32
AF = mybir.ActivationFunctionType


def build_nc(tpb: int, pb: int = PB, ncores: int = N_CORES):
    """Build the SPMD Bass program. tpb = edge tiles (of 128) per dst block."""
    np_rows = pb * 128
    tt = pb * tpb
    nc = bacc.Bacc("TRN2", target_bir_lowering=False, num_devices=ncores,
                   dynamic_dma_scratch_size=32768)

    kf = F // 128  # contraction chunks for x @ W
    # One packed constant blob (int32 columns) so every constant lands in
    # SBUF via a single DMA -> a single completion semaphore tick.
    # Layout (int32 cols): srcs[tt] | dstf[tt/2] | iota_rep[64*tpb]
    #                      | deg[pb] | bias[C] | w[kf*C/2]
    cb = tt + tt // 2 + 64 * tpb + pb + C + kf * C // 2
    xt_in = nc.dram_tensor("xt_sh", [F, np_rows], bf16, kind="ExternalInput")
    cb_in = nc.dram_tensor("cblob", [128, cb], i32, kind="ExternalInput")
    out_t = nc.dram_tensor("out", [np_rows, C], f32, kind="ExternalOutput")

    with tile.TileContext(nc) as tc, ExitStack() as ctx:
        const = ctx.enter_context(tc.tile_pool(name="const", bufs=1))
        dram = ctx.enter_context(tc.tile_pool(name="dram", bufs=1, space="DRAM"))

        # Constants / persistent state (single packed DMA)
        blob = const.tile([128, cb], i32)
        nc.sync.dma_start(out=blob[:], in_=cb_in[:, :])
        o1 = tt
        o2 = o1 + tt // 2
        o3 = o2 + 64 * tpb
        o4 = o3 + pb
        o5 = o4 + C
        srcs = blob[:, 0:o1]
        dstf = blob[:, o1:o2].bitcast(bf16)          # [128, tt]
        iota_r = blob[:, o2:o3].bitcast(bf16)        # [128, 128*tpb]
        deg_t = blob[:, o3:o4].bitcast(f32)          # [128, pb]
        bias_t = blob[:, o4:o5].bitcast(f32)         # [128, C]
        w_bf = blob[:, o5:cb].bitcast(bf16)          # [128, kf*C]

        diss = const.tile([128, pb], f32)   # deg^-1/2
        d2 = const.tile([128, pb], f32)     # deg^-1
        alph = const.tile([128, pb], f32)   # deg^-3/2
        nc.vector.reciprocal(d2[:], deg_t)
        nc.scalar.activation(diss[:], d2[:], AF.Sqrt)
        nc.vector.tensor_mul(alph[:], d2[:], diss[:])

        yself = const.tile([128, pb * C], f32)  # xw * deg^-1/2 (self-loop)

        y_sh = dram.tile([np_rows, C], bf16)
        y_full = dram.tile([ncores * np_rows, C], bf16, addr_space="Shared")

        # ---- Phase A: xw = x @ W, y = xw * diss ----
        tw = 7 if pb % 7 == 0 else (2 if pb % 2 == 0 else 1)
        xt3 = xt_in.ap().rearrange("(k p) n -> p k n", p=128)
        with (
            tc.tile_pool(name="xa", bufs=2) as xa,
            tc.tile_pool(name="psA", bufs=4, space="PSUM") as psa,
            tc.tile_pool(name="ya", bufs=2) as yap,
        ):
            for tg in range(pb // tw):
                xg = xa.tile([128, kf, tw * 128], bf16)
                nc.sync.dma_start(
                    out=xg[:],
                    in_=xt3[:, :, tg * tw * 128:(tg + 1) * tw * 128],
                )
                ybg = yap.tile([128, tw * C], bf16)
                for j in range(tw):
                    t = tg * tw + j
                    ps_xw = psa.tile([128, C], f32, tag="psxw")
                    for k in range(kf):
                        nc.tensor.matmul(
                            ps_xw[:],
                            lhsT=xg[:, k, j * 128:(j + 1) * 128],
                            rhs=w_bf[:, k * C:(k + 1) * C],
                            start=(k == 0), stop=(k == kf - 1),
                        )
                    nc.vector.tensor_scalar_mul(
                        ybg[:, j * C:(j + 1) * C], ps_xw[:], diss[:, t:t + 1]
                    )
                    nc.vector.tensor_scalar_mul(
                        yself[:, t * C:(t + 1) * C], ps_xw[:], diss[:, t:t + 1]
                    )
                nc.sync.dma_start(
                    out=y_sh[tg * tw * 128:(tg + 1) * tw * 128, :].rearrange(
                        "(g p) c -> p g c", p=128
                    ),
                    in_=ybg[:].rearrange("p (g c) -> p g c", c=C),
                )

        # ---- Phase B: replicate y across cores ----
        nc.gpsimd.collective_compute(
            "AllGather",
            mybir.AluOpType.bypass,
            replica_groups=[list(range(ncores))],
            ins=[y_sh[:].opt()],
            outs=[y_full[:].opt()],
        )

        # ---- Phase C: gather + aggregate + epilogue per dst block ----
        eg = 7 if pb % 7 == 0 else (2 if pb % 2 == 0 else 1)
        with (
            tc.tile_pool(name="gth", bufs=3) as gp,
            tc.tile_pool(name="oh", bufs=2) as ohp,
            tc.tile_pool(name="psC", bufs=4, space="PSUM") as psc,
            tc.tile_pool(name="ep", bufs=3) as ep,
            tc.tile_pool(name="og", bufs=2) as ogp,
        ):
            og = None
            for b in range(pb):
                if b % eg == 0:
                    og = ogp.tile([128, eg * C], f32)
                g = gp.tile([128, tpb * C], bf16)
                # HW indirect DMA honors exactly one index per partition per
                # instruction (verified): issue one gather per 128-edge tile.
                for t in range(tpb):
                    nc.gpsimd.indirect_dma_start(
                        out=g[:, t * C:(t + 1) * C],
                        out_offset=None,
                        in_=y_full[:, :],
                        in_offset=IndirectOffsetOnAxis(
                            ap=srcs[:, b * tpb + t:b * tpb + t + 1], axis=0
                        ),
                    )
                oh = ohp.tile([128, 128 * tpb], bf16)
                oh3 = oh[:].rearrange("p (l t) -> p l t", t=tpb)
                d3 = (
                    dstf[:, b * tpb:(b + 1) * tpb]
                    .rearrange("p (o t) -> p o t", o=1)
                    .to_broadcast([128, 128, tpb])
                )
                i3 = iota_r.rearrange("p (l t) -> p l t", t=tpb)
                nc.vector.tensor_tensor(
                    out=oh3, in0=d3, in1=i3, op=mybir.AluOpType.is_equal
                )
                pss = psc.tile([128, C], f32, tag="agg")
                for t in range(tpb):
                    nc.tensor.matmul(
                        pss[:],
                        lhsT=oh3[:, :, t],
                        rhs=g[:, t * C:(t + 1) * C],
                        start=(t == 0),
                        stop=(t == tpb - 1),
                    )
                v = ep.tile([128, C], f32, tag="v")
                nc.vector.tensor_add(v[:], pss[:], yself[:, b * C:(b + 1) * C])
                nc.vector.tensor_scalar(
                    v[:], v[:], alph[:, b:b + 1], None, op0=mybir.AluOpType.mult
                )
                nc.vector.tensor_add(v[:], v[:], bias_t)
                nm = ep.tile([128, 1], f32, tag="nm")
                nc.vector.reduce_max(
                    nm[:], v[:], axis=mybir.AxisListType.X, negate=True
                )
                ex = ep.tile([128, C], f32, tag="ex")
                z = ep.tile([128, 1], f32, tag="z")
                nc.scalar.activation(
                    ex[:], v[:], AF.Exp, bias=nm[:], scale=1.0, accum_out=z[:]
                )
                lz = ep.tile([128, 1], f32, tag="lz")
                nc.scalar.activation(lz[:], z[:], AF.Ln)
                c0 = ep.tile([128, 1], f32, tag="c0")
                nc.vector.tensor_sub(c0[:], nm[:], lz[:])
                nc.vector.tensor_scalar_add(
                    og[:, (b % eg) * C:(b % eg + 1) * C], v[:], c0[:]
                )
                if b % eg == eg - 1:
                    b0 = b - eg + 1
                    nc.sync.dma_start(
                        out=out_t[b0 * 128:(b + 1) * 128, :].rearrange(
                            "(g p) c -> p g c", p=128
                        ),
                        in_=og[:].rearrange("p (g c) -> p g c", c=C),
                    )

    nc.compile()
    return nc


def _balance_blocks(cnt, pb):
    """LPT-assign nsh nodes to pb blocks of <=128 slots, balancing total
    edge count per block. Returns slot_of[node] = block*128 + lane."""
    nsh = len(cnt)
    order = np.argsort(-cnt, kind="stable")
    heap = [(0, b) for b in range(pb)]
    heapq.heapify(heap)
    used = np.zeros(pb, dtype=np.int64)
    slot_of = np.zeros(nsh, dtype=np.int64)
    for node in order:
        tot, blk = heapq.heappop(heap)
        slot_of[node] = blk * 128 + used[blk]
        used[blk] += 1
        if used[blk] < 128:
            heapq.heappush(heap, (tot + int(cnt[node]), blk))
    return slot_of


def host_prep(x, edge_index, W, b, ncores=N_CORES, nsh=NSH, pb=PB, min_tpb=2):
    """Pure index/layout preprocessing. Returns (in_maps, tpb, slot_all)."""
    n = x.shape[0]
    np_rows = pb * 128
    src = np.asarray(edge_index[0], dtype=np.int64)
    dst = np.asarray(edge_index[1], dtype=np.int64)

    deg = (np.bincount(dst, minlength=n) + 1).astype(np.float32)  # + self loop
    ecnt = np.bincount(dst, minlength=n).astype(np.int64)

    # Balanced dst-node -> (block, lane) assignment per core.
    slot_all = np.zeros((ncores, nsh), dtype=np.int64)
    for c in range(ncores):
        slot_all[c] = _balance_blocks(ecnt[c * nsh:(c + 1) * nsh], pb)

    dslot = slot_all[dst // nsh, dst % nsh]
    core_e = dst // nsh
    blk_e = dslot // 128
    lane_e = dslot % 128
    g2p = ((src // nsh) * np_rows + slot_all[src // nsh, src % nsh]).astype(
        np.int32
    )

    gid = core_e * pb + blk_e
    counts = np.bincount(gid, minlength=ncores * pb)
    tpb = max(min_tpb, int(np.ceil(counts.max() / 128)))
    tpb += tpb & 1  # keep even for int32 blob packing
    tt = pb * tpb

    order = np.argsort(gid, kind="stable")
    s_gid = gid[order]
    group_start = np.zeros(ncores * pb, dtype=np.int64)
    np.cumsum(counts[:-1], out=group_start[1:])
    pos = np.arange(len(dst), dtype=np.int64) - group_start[s_gid]
    tile_j = pos // 128
    part_p = pos % 128
    col = blk_e[order] * tpb + tile_j

    src_arr = np.zeros((ncores, 128, tt), dtype=np.int32)
    dst_arr = np.full((ncores, 128, tt), -1.0, dtype=np.float32)
    src_arr[core_e[order], part_p, col] = g2p[order]
    dst_arr[core_e[order], part_p, col] = lane_e[order].astype(np.float32)
    dst_arr = dst_arr.astype(ml_dtypes.bfloat16)

    iota_rep = np.broadcast_to(
        np.repeat(np.arange(128, dtype=np.float32), tpb), (128, 128 * tpb)
    ).astype(ml_dtypes.bfloat16).copy()
    bias_rep = np.broadcast_to(
        np.asarray(b, dtype=np.float32), (128, C)
    ).astype(np.float32).copy()
    kf = F // 128
    w_arr = np.ascontiguousarray(
        np.asarray(W, dtype=np.float32)
        .reshape(kf, 128, C)
        .transpose(1, 0, 2)
        .astype(ml_dtypes.bfloat16)
    ).reshape(128, kf * C)
    x_bf = np.asarray(x, dtype=np.float32).astype(ml_dtypes.bfloat16)

    in_maps = []
    for c in range(ncores):
        xt_sh = np.zeros((F, np_rows), dtype=ml_dtypes.bfloat16)
        xt_sh[:, slot_all[c]] = x_bf[c * nsh:(c + 1) * nsh].T
        deg_slot = np.ones(np_rows, dtype=np.float32)
        deg_slot[slot_all[c]] = deg[c * nsh:(c + 1) * nsh]
        deg_sh = np.ascontiguousarray(deg_slot.reshape(pb, 128).T)
        blob = np.concatenate(
            [
                src_arr[c].view(np.uint8),
                dst_arr[c].view(np.uint8),
                iota_rep.view(np.uint8),
                deg_sh.view(np.uint8),
                bias_rep.view(np.uint8),
                w_arr.view(np.uint8),
            ],
            axis=1,
        ).view(np.int32)
        in_maps.append({"xt_sh": xt_sh, "cblob": blob})
    return in_maps, tpb, slot_all


def run(x, edge_index, W, b, trace=False, **spmd_kwargs):
    in_maps, tpb, slot_all = host_prep(x, edge_index, W, b)
    nc = build_nc(tpb)
    res = bass_utils.run_bass_kernel_spmd(
        nc, in_maps, core_ids=list(range(N_CORES)), trace=trace, **spmd_kwargs
    )
    out = np.concatenate(
        [res.results[c]["out"][slot_all[c]] for c in range(N_CORES)], axis=0
    )
    return out, res


def kernel(x, edge_index, W, b):
    out, _ = run(x, edge_index, W, b)
    return out

